# revision 28
# baseline (speedup 1.0000x reference)
"""Trainium2 Bass kernel for a 2-layer LSTM forecaster (nn_RNN_25177098289484).

Reference computation (per batch row b of B=16384):
    u_t   = W_in @ x_t + b_in                  (input projection, LAG=5 -> H=512)
    layer0 LSTM (H=512), layer1 LSTM (H=512), 10 steps
    outputs[:, t] = W_out @ h1_t + b_out       (per-step scalar head)
    outputs[:, 10:] = W_f @ h1_9 + b_f         (forecast head, F=5)
    returns (outputs [B,15], h_n [2,B,512], c_n [2,B,512])

Strategy: data-parallel over 8 NeuronCores (batch shard BS=2048/core).
Per core, the batch shard is processed as two independent sequential
half-scans of BH=1024 columns (fits SBUF).  Everything is kept in a
"transposed" layout [feature, batch] on chip so the recurrent matmuls
need no per-step transposes:

    g0 = W_eff @ x_t^T + Whh0 @ h0^T       (W_eff = Wih0 @ W_in folded on host)
    g1 = Wih1 @ h0^T + Whh1 @ h1^T         (accumulated in one PSUM group)

Matmuls run in float32r (full-rate fp32 with reduced internal mantissa,
measured max-rel-err ~1.6e-4 for K=512).  Gate activations run on ScalarE
(sigmoid/tanh share one table set) with the folded per-partition biases
fused into the ACTIVATE.  Cell math runs on VectorE in fp32; c streams
through DRAM, h (both layers) is ping-pong resident in SBUF as f32r.
Recurrent weights stream from HBM per (step, m-tile).  Final h/c states
are transposed back to [batch, H] with TensorE transposes at the tail.
"""

import os

import numpy as np

import concourse.bass as bass
import concourse.mybir as mybir
import concourse.tile as tile
from concourse import bacc
from concourse.bass_utils import run_bass_kernel_spmd
from concourse.masks import make_identity

S, B, LAG, H, F = 10, 16384, 5, 512, 5
NCORES = 8
BS = B // NCORES          # 2048 batch columns per core
HALVES = 2
BH = BS // HALVES         # 1024 columns per half-scan
KT = H // 128             # 4 contraction chunks for H
MT = 4 * H // 128         # 16 gate row-tiles (4H)
NH = BH // 512            # 2 N=512 column chunks per psum tile
F32 = mybir.dt.float32
F32R = mybir.dt.float32r
SIG = mybir.ActivationFunctionType.Sigmoid
TANH = mybir.ActivationFunctionType.Tanh
IDENT = mybir.ActivationFunctionType.Identity

LAST_EXEC_TIME_NS = None


def _build_program():
    nc = bacc.Bacc("TRN2", target_bir_lowering=False, debug=False)

    xT = nc.dram_tensor("xT", [S, LAG, BS], F32R, kind="ExternalInput")
    WeT = nc.dram_tensor("WeT", [LAG, 4 * H], F32R, kind="ExternalInput")
    W0T = nc.dram_tensor("W0T", [H, 4 * H], F32R, kind="ExternalInput")
    W1T = nc.dram_tensor("W1T", [2 * H, 4 * H], F32R, kind="ExternalInput")
    WofT = nc.dram_tensor("WofT", [H, 8], F32R, kind="ExternalInput")
    BIAS0 = nc.dram_tensor("BIAS0", [128, MT], F32, kind="ExternalInput")
    BIAS1 = nc.dram_tensor("BIAS1", [128, MT], F32, kind="ExternalInput")
    BOF = nc.dram_tensor("BOF", [6, 1], F32, kind="ExternalInput")

    OUT = nc.dram_tensor("OUT", [BS, S + F], F32, kind="ExternalOutput")
    HN = nc.dram_tensor("HN", [2, BS, H], F32, kind="ExternalOutput")
    CN = nc.dram_tensor("CN", [2, BS, H], F32, kind="ExternalOutput")

    # internal DRAM: c state stream, layout [layer, k, p, b]
    CT = nc.dram_tensor("cT", [2, KT, 128, BS], F32)

    w0r = W0T.ap().rearrange("(k p) m -> p k m", p=128)
    w1r = W1T.ap().rearrange("(k p) m -> p k m", p=128)

    with tile.TileContext(nc) as tc:
        with (
            tc.tile_pool(name="const", bufs=1) as constp,
            tc.tile_pool(name="state", bufs=1) as statep,
        ):
            ident = constp.tile([128, 128], F32, tag="ident")
            make_identity(nc, ident[:])
            # W_eff^T replicated at partition bases 0/32/64/96 so the four
            # rank-5 x-projection matmuls of a quad can run concurrently in
            # distinct PE row-groups (tile_position row tiling)
            wet = constp.tile([128, 4 * H], F32R, tag="wet")
            for i in range(2):
                nc.sync.dma_start(wet[32 * i:32 * i + LAG, :], WeT.ap())
            woft = constp.tile([128, KT, 8], F32R, tag="woft")
            nc.sync.dma_start(
                woft[:, :, 0:6], WofT.ap().rearrange("(k p) c -> p k c", p=128)[:, :, 0:6]
            )
            bias0 = constp.tile([128, MT], F32, tag="bias0")
            nc.sync.dma_start(bias0[:], BIAS0.ap())
            bias1 = constp.tile([128, MT], F32, tag="bias1")
            nc.sync.dma_start(bias1[:], BIAS1.ap())
            bof = constp.tile([6, 1], F32, tag="bof")
            nc.sync.dma_start(bof[:], BOF.ap())

            head_sb = statep.tile([32, BS], F32, tag="head_sb")
            hst = [
                statep.tile([128, KT, BH], F32R, tag=f"h{i}", name=f"h{i}") for i in range(4)
            ]  # h0a h0b h1a h1b

            with (
                tc.tile_pool(name="w0p", bufs=6) as w0p,
                tc.tile_pool(name="w1p", bufs=6) as w1p,
                tc.tile_pool(name="xp", bufs=2) as xp,
                tc.tile_pool(name="actp", bufs=8) as actp,
                tc.tile_pool(name="cinp", bufs=4) as cinp,
                tc.tile_pool(name="cnewp", bufs=4) as cnewp,
                tc.tile_pool(name="rowp", bufs=2) as rowp,
                tc.tile_pool(name="vtp", bufs=2) as vtp,
                tc.tile_pool(name="gpsum", bufs=3, space="PSUM") as gpsum,
                tc.tile_pool(name="hpsum", bufs=2, space="PSUM") as hpsum,
            ):
                def emit_xmm(ps, ri, m, xt, t):
                    # rank-5 W_eff @ x_t — the two pair members run
                    # concurrently in distinct 32-row PE groups
                    for nh in range(NH):
                        nsl = slice(nh * 512, (nh + 1) * 512)
                        nc.tensor.matmul(
                            ps[:, nsl],
                            wet[32 * ri:32 * ri + LAG, m * 128:(m + 1) * 128],
                            xt[32 * ri:32 * ri + LAG, nsl],
                            start=True,
                            stop=(t == 0),
                            tile_position=(32 * ri, 0),
                        )

                def emit_hh(ps, wm, rhs_sets, layer):
                    first = layer == 1
                    n_sets = len(rhs_sets)
                    for si, (rhs, kofs) in enumerate(rhs_sets):
                        for kk in range(KT):
                            last = si == n_sets - 1 and kk == KT - 1
                            for nh in range(NH):
                                nsl = slice(nh * 512, (nh + 1) * 512)
                                nc.tensor.matmul(
                                    ps[:, nsl],
                                    wm[:, kofs + kk, :],
                                    rhs[:, kk, nsl],
                                    start=(first and kk == 0),
                                    stop=last,
                                )
                        first = False

                def state_out(val, dst, layer, k, c0):
                    # final states: 32x32 block-transpose on DVE, then a
                    # block-permuting DMA straight into the [batch, H]
                    # outputs — no PE tail needed
                    vt = vtp.tile([128, BH], F32, name="vt")
                    nc.vector.transpose(vt[:], val)
                    for a in range(4):
                        dpc = dst.ap()[
                            layer, c0:c0 + BH,
                            k * 128 + 32 * a:k * 128 + 32 * (a + 1),
                        ].rearrange("(c j) i -> j c i", j=32)
                        spc = vt[32 * a:32 * (a + 1), :].rearrange(
                            "j (c i) -> j c i", i=32
                        )
                        nc.gpsimd.dma_start(dpc, spc)

                def cell(t, c0, layer, k, acts, hdst, cin):
                    a_i, a_f = acts[k], acts[4 + k]
                    a_g, a_o = acts[8 + k], acts[12 + k]
                    cnew = cnewp.tile([128, BH], F32)
                    if t == 0:
                        nc.vector.tensor_mul(cnew[:], a_i[:], a_g[:])
                    else:
                        nc.vector.tensor_mul(a_f[:], a_f[:], cin[:])
                        nc.vector.tensor_mul(a_i[:], a_i[:], a_g[:])
                        nc.vector.tensor_add(cnew[:], a_f[:], a_i[:])
                    if t < S - 1:
                        nc.gpsimd.dma_start(
                            CT.ap()[layer, k, :, c0:c0 + BH], cnew[:]
                        )
                    nc.scalar.activation(a_g[:], cnew[:], TANH)
                    nc.vector.tensor_mul(hdst[:, k, :], a_o[:], a_g[:])
                    if t == S - 1:
                        if layer == 0:
                            state_out(cnew[:], CN, layer, k, c0)
                            state_out(
                                hdst[:, k, :].bitcast(F32), HN, layer, k, c0
                            )
                        else:
                            # defer layer-1 state outputs so the final head
                            # matmuls aren't stuck behind them in the
                            # in-order DVE stream
                            pending_outs.append((cnew, hdst, k, c0))

                def quad_step(t, c0, layer, k, rhs_sets, xt, biast, wsrc,
                              wpool, kchunks, hdst):
                    quad = (k, 4 + k, 8 + k, 12 + k)
                    cin = None
                    if t > 0:
                        # prefetch this quad's c chunk a full quad early
                        cin = cinp.tile([128, BH], F32, name="cin")
                        nc.sync.dma_start(
                            cin[:], CT.ap()[layer, k, :, c0:c0 + BH]
                        )
                    wtiles = {}
                    for m in quad:
                        wm = wpool.tile([128, kchunks, 128], F32R, name="wm")
                        nc.sync.dma_start(
                            wm[:], wsrc[:, :, m * 128:(m + 1) * 128]
                        )
                        wtiles[m] = wm
                    acts = {}
                    for pair in ((k, 8 + k), (4 + k, 12 + k)):
                        pss = {}
                        for ri, m in enumerate(pair):
                            ps = gpsum.tile([128, BH], F32, name="gps")
                            pss[m] = ps
                            if layer == 0:
                                emit_xmm(ps, ri, m, xt, t)
                        for m in pair:
                            ps = pss[m]
                            emit_hh(ps, wtiles[m], rhs_sets, layer)
                            act = actp.tile([128, BH], F32, name="act")
                            func = TANH if 8 <= m < 12 else SIG
                            nc.scalar.activation(
                                act[:], ps[:], func, bias=biast[:, m:m + 1]
                            )
                            acts[m] = act
                    cell(t, c0, layer, k, acts, hdst, cin)

                pending_outs = []
                for half in range(HALVES):
                    c0 = half * BH

                    def head(t, h1_tile, Mh):
                        row = rowp.tile([6, BH], F32)
                        for nh in range(NH):
                            ps = hpsum.tile([6, 512], F32)
                            for kk in range(KT):
                                nc.tensor.matmul(
                                    ps[0:Mh, :],
                                    woft[:, kk, 0:Mh],
                                    h1_tile[:, kk, nh * 512:(nh + 1) * 512],
                                    start=(kk == 0),
                                    stop=(kk == KT - 1),
                                )
                            nc.scalar.activation(
                                row[0:Mh, nh * 512:(nh + 1) * 512], ps[0:Mh, :],
                                IDENT, bias=bof[0:Mh, 0:1],
                            )
                        nc.gpsimd.dma_start(
                            head_sb[t:t + Mh, c0:c0 + BH], row[0:Mh, :]
                        )

                    for t in range(S):
                        h0_cur, h0_nxt = hst[t % 2], hst[(t + 1) % 2]
                        h1_cur, h1_nxt = hst[2 + t % 2], hst[2 + (t + 1) % 2]

                        xt = xp.tile([128, BH], F32R)
                        for i in range(2):
                            nc.sync.dma_start(
                                xt[32 * i:32 * i + LAG, :],
                                xT.ap()[t, :, c0:c0 + BH],
                            )
                        # head for previous step (h1_cur is h1(t-1)) — emitted
                        # early so PE fills the cell-latency gap
                        if t > 0:
                            head(t - 1, h1_cur, 1)

                        for layer in range(2):
                            if layer == 0:
                                rhs_sets = [] if t == 0 else [(h0_cur, 0)]
                                args = (bias0, w0r, w0p, KT, h0_nxt)
                            else:
                                # h1(t-1) chunks first (ready early), then
                                # h0(t) chunks
                                rhs_sets = (
                                    [(h0_nxt, 0)]
                                    if t == 0
                                    else [(h1_cur, KT), (h0_nxt, 0)]
                                )
                                args = (bias1, w1r, w1p, 2 * KT, h1_nxt)
                            for k in range(KT):
                                quad_step(t, c0, layer, k, rhs_sets, xt, *args)

                    # final-step head (M=6 covers step-9 + forecast rows)
                    hfin = [hst[S % 2], hst[2 + S % 2]]
                    head(S - 1, hfin[1], 6)
                    for cnew, hdst, k, cc0 in pending_outs:
                        state_out(cnew[:], CN, 1, k, cc0)
                        state_out(hdst[:, k, :].bitcast(F32), HN, 1, k, cc0)
                    pending_outs.clear()

            # ---- tail: head block [15, BS] -> OUT [BS, 15] via DVE 32x32
            # block transpose + block-permuting DMA (no PE involved) ----
            with tc.tile_pool(name="htail", bufs=1) as htailp:
                vt = htailp.tile([32, BS], F32, tag="htail")
                nc.vector.transpose(vt[:], head_sb[:])
                dpc = OUT.ap().rearrange("(c j) f -> j c f", j=32)
                spc = vt[:].rearrange("j (c i) -> j c i", i=32)[:, :, 0:15]
                nc.gpsimd.dma_start(dpc, spc)

    nc.compile()
    return nc


_PROGRAM = None


def kernel(x, W_in, b_in, Wih0, Whh0, bih0, bhh0, Wih1, Whh1, bih1, bhh1,
           W_out, b_out, W_f, b_f):
    global _PROGRAM, LAST_EXEC_TIME_NS
    x = np.asarray(x, np.float32)

    # fold the input projection into layer-0's input-side gate weights
    WeT = np.ascontiguousarray(
        (np.asarray(Wih0, np.float32) @ np.asarray(W_in, np.float32)).T
    )  # [LAG, 4H]
    beff0 = (
        np.asarray(Wih0, np.float32) @ np.asarray(b_in, np.float32)
        + np.asarray(bih0, np.float32) + np.asarray(bhh0, np.float32)
    )
    beff1 = np.asarray(bih1, np.float32) + np.asarray(bhh1, np.float32)
    W0T = np.ascontiguousarray(np.asarray(Whh0, np.float32).T)      # [H, 4H]
    W1T = np.ascontiguousarray(
        np.concatenate(
            [np.asarray(Wih1, np.float32).T, np.asarray(Whh1, np.float32).T], axis=0
        )
    )  # [2H, 4H]
    WofT = np.zeros((H, 8), np.float32)
    WofT[:, 0:1] = np.asarray(W_out, np.float32).T
    WofT[:, 1:6] = np.asarray(W_f, np.float32).T
    BIAS0 = np.ascontiguousarray(beff0.reshape(MT, 128).T)
    BIAS1 = np.ascontiguousarray(beff1.reshape(MT, 128).T)
    BOF = np.concatenate(
        [np.asarray(b_out, np.float32), np.asarray(b_f, np.float32)]
    ).reshape(6, 1)

    if _PROGRAM is None:
        _PROGRAM = _build_program()
    nc = _PROGRAM

    shared = {
        "WeT": WeT, "W0T": W0T, "W1T": W1T, "WofT": WofT,
        "BIAS0": BIAS0, "BIAS1": BIAS1, "BOF": BOF,
    }
    in_maps = []
    for c in range(NCORES):
        xs = np.ascontiguousarray(
            x[:, c * BS:(c + 1) * BS, :].transpose(0, 2, 1)
        )  # [S, LAG, BS]
        in_maps.append({"xT": xs, **shared})

    trace = os.environ.get("TRN_KERNEL_TRACE", "0") == "1"
    res = run_bass_kernel_spmd(
        nc, in_maps, core_ids=list(range(NCORES)), trace=trace
    )
    LAST_EXEC_TIME_NS = res.exec_time_ns

    outputs = np.concatenate([r["OUT"] for r in res.results], axis=0)
    h_n = np.concatenate([r["HN"] for r in res.results], axis=1)
    c_n = np.concatenate([r["CN"] for r in res.results], axis=1)
    return outputs, h_n, c_n


# revision 29
# speedup vs baseline: 1.0453x; 1.0453x over previous
"""Trainium2 Bass kernel for a 2-layer LSTM forecaster (nn_RNN_25177098289484).

Reference computation (per batch row b of B=16384):
    u_t   = W_in @ x_t + b_in                  (input projection, LAG=5 -> H=512)
    layer0 LSTM (H=512), layer1 LSTM (H=512), 10 steps
    outputs[:, t] = W_out @ h1_t + b_out       (per-step scalar head)
    outputs[:, 10:] = W_f @ h1_9 + b_f         (forecast head, F=5)
    returns (outputs [B,15], h_n [2,B,512], c_n [2,B,512])

Strategy: data-parallel over 8 NeuronCores (batch shard BS=2048/core).
Per core, the batch shard is processed as two independent sequential
half-scans of BH=1024 columns (fits SBUF).  Everything is kept in a
"transposed" layout [feature, batch] on chip so the recurrent matmuls
need no per-step transposes:

    g0 = W_eff @ x_t^T + Whh0 @ h0^T       (W_eff = Wih0 @ W_in folded on host)
    g1 = Wih1 @ h0^T + Whh1 @ h1^T         (accumulated in one PSUM group)

Matmuls run in float32r (full-rate fp32 with reduced internal mantissa,
measured max-rel-err ~1.6e-4 for K=512).  Gate activations run on ScalarE
(sigmoid/tanh share one table set) with the folded per-partition biases
fused into the ACTIVATE.  Cell math runs on VectorE in fp32; c streams
through DRAM, h (both layers) is ping-pong resident in SBUF as f32r.
Recurrent weights stream from HBM per (step, m-tile).  Final h/c states
are transposed back to [batch, H] with TensorE transposes at the tail.
"""

import os

import numpy as np

import concourse.bass as bass
import concourse.mybir as mybir
import concourse.tile as tile
from concourse import bacc
from concourse.bass_utils import run_bass_kernel_spmd
from concourse.masks import make_identity

S, B, LAG, H, F = 10, 16384, 5, 512, 5
NCORES = 8
BS = B // NCORES          # 2048 batch columns per core
HALVES = 2
BH = BS // HALVES         # 1024 columns per half-scan
KT = H // 128             # 4 contraction chunks for H
MT = 4 * H // 128         # 16 gate row-tiles (4H)
NH = BH // 512            # 2 N=512 column chunks per psum tile
F32 = mybir.dt.float32
F32R = mybir.dt.float32r
SIG = mybir.ActivationFunctionType.Sigmoid
TANH = mybir.ActivationFunctionType.Tanh
IDENT = mybir.ActivationFunctionType.Identity

LAST_EXEC_TIME_NS = None


def _build_program():
    nc = bacc.Bacc("TRN2", target_bir_lowering=False, debug=False)

    xT = nc.dram_tensor("xT", [S, LAG, BS], F32R, kind="ExternalInput")
    WeT = nc.dram_tensor("WeT", [LAG, 4 * H], F32R, kind="ExternalInput")
    W0T = nc.dram_tensor("W0T", [H, 4 * H], F32R, kind="ExternalInput")
    W1T = nc.dram_tensor("W1T", [2 * H, 4 * H], F32R, kind="ExternalInput")
    WofT = nc.dram_tensor("WofT", [H, 8], F32R, kind="ExternalInput")
    BIAS0 = nc.dram_tensor("BIAS0", [128, MT], F32, kind="ExternalInput")
    BIAS1 = nc.dram_tensor("BIAS1", [128, MT], F32, kind="ExternalInput")
    BOF = nc.dram_tensor("BOF", [6, 1], F32, kind="ExternalInput")

    OUT = nc.dram_tensor("OUT", [BS, S + F], F32, kind="ExternalOutput")
    HN = nc.dram_tensor("HN", [2, BS, H], F32, kind="ExternalOutput")
    CN = nc.dram_tensor("CN", [2, BS, H], F32, kind="ExternalOutput")

    # internal DRAM: c state stream, layout [layer, k, p, b]
    CT = nc.dram_tensor("cT", [2, KT, 128, BS], F32)

    w0r = W0T.ap().rearrange("(k p) m -> p k m", p=128)
    w1r = W1T.ap().rearrange("(k p) m -> p k m", p=128)

    with tile.TileContext(nc) as tc:
        with (
            tc.tile_pool(name="const", bufs=1) as constp,
            tc.tile_pool(name="state", bufs=1) as statep,
        ):
            ident = constp.tile([128, 128], F32, tag="ident")
            make_identity(nc, ident[:])
            # W_eff^T replicated at partition bases 0/32/64/96 so the four
            # rank-5 x-projection matmuls of a quad can run concurrently in
            # distinct PE row-groups (tile_position row tiling)
            wet = constp.tile([128, 4 * H], F32R, tag="wet")
            for i in range(2):
                nc.sync.dma_start(wet[32 * i:32 * i + LAG, :], WeT.ap())
            woft = constp.tile([128, KT, 8], F32R, tag="woft")
            nc.sync.dma_start(
                woft[:, :, 0:6], WofT.ap().rearrange("(k p) c -> p k c", p=128)[:, :, 0:6]
            )
            bias0 = constp.tile([128, MT], F32, tag="bias0")
            nc.sync.dma_start(bias0[:], BIAS0.ap())
            bias1 = constp.tile([128, MT], F32, tag="bias1")
            nc.sync.dma_start(bias1[:], BIAS1.ap())
            bof = constp.tile([6, 1], F32, tag="bof")
            nc.sync.dma_start(bof[:], BOF.ap())

            head_sb = statep.tile([32, BS], F32, tag="head_sb")
            hst = [
                statep.tile([128, KT, BH], F32R, tag=f"h{i}", name=f"h{i}") for i in range(4)
            ]  # h0a h0b h1a h1b

            with (
                tc.tile_pool(name="w0p", bufs=6) as w0p,
                tc.tile_pool(name="w1p", bufs=6) as w1p,
                tc.tile_pool(name="xp", bufs=2) as xp,
                tc.tile_pool(name="actp", bufs=8) as actp,
                tc.tile_pool(name="cinp", bufs=4) as cinp,
                tc.tile_pool(name="cnewp", bufs=4) as cnewp,
                tc.tile_pool(name="rowp", bufs=2) as rowp,
                tc.tile_pool(name="vtp", bufs=2) as vtp,
                tc.tile_pool(name="gpsum", bufs=3, space="PSUM") as gpsum,
                tc.tile_pool(name="hpsum", bufs=2, space="PSUM") as hpsum,
            ):
                def emit_xmm(ps, ri, m, xt, t):
                    # rank-5 W_eff @ x_t — the two pair members run
                    # concurrently in distinct 32-row PE groups
                    for nh in range(NH):
                        nsl = slice(nh * 512, (nh + 1) * 512)
                        nc.tensor.matmul(
                            ps[:, nsl],
                            wet[32 * ri:32 * ri + LAG, m * 128:(m + 1) * 128],
                            xt[32 * ri:32 * ri + LAG, nsl],
                            start=True,
                            stop=(t == 0),
                            tile_position=(32 * ri, 0),
                        )

                def emit_hh(ps, wm, rhs_sets, layer):
                    first = layer == 1
                    n_sets = len(rhs_sets)
                    for si, (rhs, kofs) in enumerate(rhs_sets):
                        for kk in range(KT):
                            last = si == n_sets - 1 and kk == KT - 1
                            for nh in range(NH):
                                nsl = slice(nh * 512, (nh + 1) * 512)
                                nc.tensor.matmul(
                                    ps[:, nsl],
                                    wm[:, kofs + kk, :],
                                    rhs[:, kk, nsl],
                                    start=(first and kk == 0),
                                    stop=last,
                                )
                        first = False

                def state_out(val, dst, layer, k, c0):
                    # final states: 32x32 block-transpose on DVE, then a
                    # block-permuting DMA straight into the [batch, H]
                    # outputs — no PE tail needed
                    vt = vtp.tile([128, BH], F32, name="vt")
                    nc.vector.transpose(vt[:], val)
                    for a in range(4):
                        dpc = dst.ap()[
                            layer, c0:c0 + BH,
                            k * 128 + 32 * a:k * 128 + 32 * (a + 1),
                        ].rearrange("(c j) i -> j c i", j=32)
                        spc = vt[32 * a:32 * (a + 1), :].rearrange(
                            "j (c i) -> j c i", i=32
                        )
                        nc.gpsimd.dma_start(dpc, spc)

                def cell(t, c0, layer, k, acts, hdst, cin):
                    a_i, a_f = acts[k], acts[4 + k]
                    a_g, a_o = acts[8 + k], acts[12 + k]
                    cnew = cnewp.tile([128, BH], F32)
                    if t == 0:
                        nc.vector.tensor_mul(cnew[:], a_i[:], a_g[:])
                    else:
                        nc.vector.tensor_mul(a_f[:], a_f[:], cin[:])
                        nc.vector.tensor_mul(a_i[:], a_i[:], a_g[:])
                        nc.vector.tensor_add(cnew[:], a_f[:], a_i[:])
                    if t < S - 1:
                        nc.gpsimd.dma_start(
                            CT.ap()[layer, k, :, c0:c0 + BH], cnew[:]
                        )
                    nc.scalar.activation(a_g[:], cnew[:], TANH)
                    nc.vector.tensor_mul(hdst[:, k, :], a_o[:], a_g[:])
                    if t == S - 1:
                        if layer == 0:
                            state_out(cnew[:], CN, layer, k, c0)
                            state_out(
                                hdst[:, k, :].bitcast(F32), HN, layer, k, c0
                            )
                        else:
                            # defer layer-1 state outputs so the final head
                            # matmuls aren't stuck behind them in the
                            # in-order DVE stream
                            pending_outs.append((cnew, hdst, k, c0))

                def quad_step(t, c0, layer, k, rhs_sets, xt, biast, wsrc,
                              wpool, kchunks, hdst):
                    quad = (k, 4 + k, 8 + k, 12 + k)
                    wtiles = {}
                    for m in quad:
                        wm = wpool.tile([128, kchunks, 128], F32R, name="wm")
                        nc.sync.dma_start(
                            wm[:], wsrc[:, :, m * 128:(m + 1) * 128]
                        )
                        wtiles[m] = wm
                    cin = None
                    if t > 0:
                        # prefetch this quad's c chunk ahead of the cell
                        cin = cinp.tile([128, BH], F32, name="cin")
                        nc.sync.dma_start(
                            cin[:], CT.ap()[layer, k, :, c0:c0 + BH]
                        )
                    acts = {}
                    for pair in ((k, 8 + k), (4 + k, 12 + k)):
                        pss = {}
                        for ri, m in enumerate(pair):
                            ps = gpsum.tile([128, BH], F32, name="gps")
                            pss[m] = ps
                            if layer == 0:
                                emit_xmm(ps, ri, m, xt, t)
                        for m in pair:
                            ps = pss[m]
                            emit_hh(ps, wtiles[m], rhs_sets, layer)
                            act = actp.tile([128, BH], F32, name="act")
                            func = TANH if 8 <= m < 12 else SIG
                            nc.scalar.activation(
                                act[:], ps[:], func, bias=biast[:, m:m + 1]
                            )
                            acts[m] = act
                    cell(t, c0, layer, k, acts, hdst, cin)

                pending_outs = []
                for half in range(HALVES):
                    c0 = half * BH

                    def head(t, h1_tile, Mh):
                        row = rowp.tile([6, BH], F32)
                        for nh in range(NH):
                            ps = hpsum.tile([6, 512], F32)
                            for kk in range(KT):
                                nc.tensor.matmul(
                                    ps[0:Mh, :],
                                    woft[:, kk, 0:Mh],
                                    h1_tile[:, kk, nh * 512:(nh + 1) * 512],
                                    start=(kk == 0),
                                    stop=(kk == KT - 1),
                                )
                            nc.scalar.activation(
                                row[0:Mh, nh * 512:(nh + 1) * 512], ps[0:Mh, :],
                                IDENT, bias=bof[0:Mh, 0:1],
                            )
                        nc.gpsimd.dma_start(
                            head_sb[t:t + Mh, c0:c0 + BH], row[0:Mh, :]
                        )

                    for t in range(S):
                        h0_cur, h0_nxt = hst[t % 2], hst[(t + 1) % 2]
                        h1_cur, h1_nxt = hst[2 + t % 2], hst[2 + (t + 1) % 2]

                        xt = xp.tile([128, BH], F32R)
                        for i in range(2):
                            nc.sync.dma_start(
                                xt[32 * i:32 * i + LAG, :],
                                xT.ap()[t, :, c0:c0 + BH],
                            )
                        # head for previous step (h1_cur is h1(t-1)) — emitted
                        # early so PE fills the cell-latency gap
                        if t > 0:
                            head(t - 1, h1_cur, 1)

                        for layer in range(2):
                            if layer == 0:
                                rhs_sets = [] if t == 0 else [(h0_cur, 0)]
                                args = (bias0, w0r, w0p, KT, h0_nxt)
                            else:
                                # h1(t-1) chunks first (ready early), then
                                # h0(t) chunks
                                rhs_sets = (
                                    [(h0_nxt, 0)]
                                    if t == 0
                                    else [(h1_cur, KT), (h0_nxt, 0)]
                                )
                                args = (bias1, w1r, w1p, 2 * KT, h1_nxt)
                            for k in range(KT):
                                quad_step(t, c0, layer, k, rhs_sets, xt, *args)

                    # final-step head (M=6 covers step-9 + forecast rows)
                    hfin = [hst[S % 2], hst[2 + S % 2]]
                    head(S - 1, hfin[1], 6)
                    for cnew, hdst, k, cc0 in pending_outs:
                        state_out(cnew[:], CN, 1, k, cc0)
                        state_out(hdst[:, k, :].bitcast(F32), HN, 1, k, cc0)
                    pending_outs.clear()

            # ---- tail: head block [15, BS] -> OUT [BS, 15] via DVE 32x32
            # block transpose + block-permuting DMA (no PE involved) ----
            with tc.tile_pool(name="htail", bufs=1) as htailp:
                vt = htailp.tile([32, BS], F32, tag="htail")
                nc.vector.transpose(vt[:], head_sb[:])
                dpc = OUT.ap().rearrange("(c j) f -> j c f", j=32)
                spc = vt[:].rearrange("j (c i) -> j c i", i=32)[:, :, 0:15]
                nc.gpsimd.dma_start(dpc, spc)

    nc.compile()
    return nc


_PROGRAM = None


def kernel(x, W_in, b_in, Wih0, Whh0, bih0, bhh0, Wih1, Whh1, bih1, bhh1,
           W_out, b_out, W_f, b_f):
    global _PROGRAM, LAST_EXEC_TIME_NS
    x = np.asarray(x, np.float32)

    # fold the input projection into layer-0's input-side gate weights
    WeT = np.ascontiguousarray(
        (np.asarray(Wih0, np.float32) @ np.asarray(W_in, np.float32)).T
    )  # [LAG, 4H]
    beff0 = (
        np.asarray(Wih0, np.float32) @ np.asarray(b_in, np.float32)
        + np.asarray(bih0, np.float32) + np.asarray(bhh0, np.float32)
    )
    beff1 = np.asarray(bih1, np.float32) + np.asarray(bhh1, np.float32)
    W0T = np.ascontiguousarray(np.asarray(Whh0, np.float32).T)      # [H, 4H]
    W1T = np.ascontiguousarray(
        np.concatenate(
            [np.asarray(Wih1, np.float32).T, np.asarray(Whh1, np.float32).T], axis=0
        )
    )  # [2H, 4H]
    WofT = np.zeros((H, 8), np.float32)
    WofT[:, 0:1] = np.asarray(W_out, np.float32).T
    WofT[:, 1:6] = np.asarray(W_f, np.float32).T
    BIAS0 = np.ascontiguousarray(beff0.reshape(MT, 128).T)
    BIAS1 = np.ascontiguousarray(beff1.reshape(MT, 128).T)
    BOF = np.concatenate(
        [np.asarray(b_out, np.float32), np.asarray(b_f, np.float32)]
    ).reshape(6, 1)

    if _PROGRAM is None:
        _PROGRAM = _build_program()
    nc = _PROGRAM

    shared = {
        "WeT": WeT, "W0T": W0T, "W1T": W1T, "WofT": WofT,
        "BIAS0": BIAS0, "BIAS1": BIAS1, "BOF": BOF,
    }
    in_maps = []
    for c in range(NCORES):
        xs = np.ascontiguousarray(
            x[:, c * BS:(c + 1) * BS, :].transpose(0, 2, 1)
        )  # [S, LAG, BS]
        in_maps.append({"xT": xs, **shared})

    trace = os.environ.get("TRN_KERNEL_TRACE", "0") == "1"
    res = run_bass_kernel_spmd(
        nc, in_maps, core_ids=list(range(NCORES)), trace=trace
    )
    LAST_EXEC_TIME_NS = res.exec_time_ns

    outputs = np.concatenate([r["OUT"] for r in res.results], axis=0)
    h_n = np.concatenate([r["HN"] for r in res.results], axis=1)
    c_n = np.concatenate([r["CN"] for r in res.results], axis=1)
    return outputs, h_n, c_n
